# revision 9
# baseline (speedup 1.0000x reference)
"""Trainium2 Bass kernel for nn_Neuromorphizer (event-camera emulator).

The reference lax.scan collapses exactly to an elementwise op per frame:
with REFRACTORY_US=0 and THRESHOLD=0, `idle` is always true (ts <= t <=
min_time), so state becomes simply the previous frame and timesurface is
dead.  Per frame t (0-indexed), per pixel:

    d    = (tensor[t] - prev) + nb[(t+1) % 10]
    out  = 0 if d < 0, 127 if d == 0, 255 if d > 0

where prev = tensor[t-1] (or `state` for t=0) and nb = +B for on-noise,
-2B for off-noise (off wins), 0 otherwise, with B chosen to dominate any
real |diff| (<= 512).  The classification is one ScalarE op,
Relu(BIGSCALE*d + 127), written straight to uint8: the f32->u8 cast
saturates on HW (verified), so d>0 maps to 255, d==0 to exactly 127,
d<0 to 0.  The subtract is exact in f32; its bf16 rounding preserves
sign and zeroness (smallest nonzero |diff| of the uniform*255 grid is
~7.6e-6, far above bf16 underflow).

Sharding: H=720 rows split across 8 cores (90 rows each, no cross-core
communication).  Per-core HBM layout is partition-major [128, T*900] so
a 10-frame chunk DMA moves 36 KB contiguous per partition.  Chunks of
10 frames align exactly with the noise period, so every chunk's noise
bias is the same [128, 10*900] plane buffer (host pre-orders planes by
(idx+1)%10).  Within a chunk the per-frame "previous frame" operand is
the chunk itself shifted one frame; only the chunk's first frame needs
the previous chunk's last frame.
"""

import sys

for _p in ("/opt/trn_rl_repo", "/opt/pypackages"):
    if _p not in sys.path:
        sys.path.append(_p)

import numpy as np

import concourse.bacc as bacc
from concourse import mybir
from concourse.tile import TileContext
from concourse.bass_utils import run_bass_kernel_spmd

T, H, W = 96, 720, 1280
N_CORES = 8
ROWS = H // N_CORES          # 90 rows per core
NPIX = ROWS * W              # 115200
P = 128                      # SBUF partitions
FD = NPIX // P               # 900 free-dim elements per frame
N_NOISE = 10
K = 10                       # frames per chunk (== noise period)

F32 = mybir.dt.float32
BF16 = mybir.dt.bfloat16
U8 = mybir.dt.uint8

NB_ON = 65536.0              # on-noise bias (bf16-exact, dominates |diff|<=512)
NB_OFF = 131072.0            # off-noise bias magnitude (off wins: -2B+B < 0)
BIGSCALE = 1.0e12            # maps smallest nonzero |d| (~7e-6) far above 255

Alu = mybir.AluOpType
Act = mybir.ActivationFunctionType


HK = 5  # frames per half-chunk; two half-chunks share one output DMA


def build_nc(frames: int = T, fd: int = FD):
    nc = bacc.Bacc(debug=False)

    x = nc.dram_tensor("x", [P, frames * fd], F32, kind="ExternalInput")
    st = nc.dram_tensor("state", [P, fd], F32, kind="ExternalInput")
    nbm = nc.dram_tensor("nb_m", [P, N_NOISE * fd], BF16, kind="ExternalInput")
    y = nc.dram_tensor("y", [P, frames * fd], U8, kind="ExternalOutput")

    hchunks = [
        (h * HK, min(HK, frames - h * HK)) for h in range((frames + HK - 1) // HK)
    ]

    with TileContext(nc) as tc:
        with (
            tc.tile_pool(name="const", bufs=1) as cpool,
            tc.tile_pool(name="frames", bufs=4) as fpool,
            tc.tile_pool(name="work", bufs=4) as wpool,
            tc.tile_pool(name="outp", bufs=2) as opool,
        ):
            # half-chunk 0's input load (2+3 frames) leads the program so
            # compute starts as early as possible
            cur0 = fpool.tile([P, HK * fd], F32, name="cur", tag="cur")
            nc.sync.dma_start(cur0[:, : 2 * fd], x[:, : 2 * fd])
            nc.sync.dma_start(cur0[:, 2 * fd :], x[:, 2 * fd : HK * fd])

            # constants: host-precomputed noise bias planes, state, bias
            nb = cpool.tile([P, N_NOISE * fd], BF16, name="nb")
            nc.sync.dma_start(nb[:], nbm[:])
            stile = cpool.tile([P, fd], F32, name="stile")
            nc.sync.dma_start(stile[:], st[:])
            bias127 = cpool.tile([P, 1], F32, name="bias127")
            nc.gpsimd.memset(bias127[:], 127.0)

            prev_last = stile[:, :]
            out = None
            out_f0 = out_w = 0
            for h, (f0, k) in enumerate(hchunks):
                cfd = k * fd
                if h == 0:
                    cur = cur0
                else:
                    cur = fpool.tile([P, HK * fd], F32, name="cur", tag="cur")
                    nc.sync.dma_start(
                        cur[:, :cfd], x[:, f0 * fd : f0 * fd + cfd]
                    )

                d = wpool.tile([P, HK * fd], BF16, name="d")
                # frame f0: cur[0] - prev_last ; rest: shifted self
                nc.vector.tensor_tensor(
                    d[:, :fd], cur[:, :fd], prev_last, Alu.subtract
                )
                if h == 0:
                    # split along the 2+3 first-load pieces
                    nc.vector.tensor_tensor(
                        d[:, fd : 2 * fd], cur[:, fd : 2 * fd], cur[:, :fd],
                        Alu.subtract,
                    )
                    nc.vector.tensor_tensor(
                        d[:, 2 * fd : cfd], cur[:, 2 * fd : cfd],
                        cur[:, fd : cfd - fd], Alu.subtract,
                    )
                elif k > 1:
                    nc.vector.tensor_tensor(
                        d[:, fd:cfd], cur[:, fd:cfd], cur[:, : cfd - fd],
                        Alu.subtract,
                    )
                # noise-bias window: half-chunks alternate nb positions 0-4/5-9
                npos = (f0 % N_NOISE) * fd
                nc.vector.tensor_tensor(
                    d[:, :cfd], d[:, :cfd], nb[:, npos : npos + cfd], Alu.add
                )
                if h % 2 == 0:
                    out = opool.tile([P, 2 * HK * fd], U8, name="out")
                    out_f0, out_w = f0, 0
                # classify: saturating u8 cast of Relu(BIGSCALE*d + 127)
                nc.scalar.activation(
                    out[:, out_w : out_w + cfd], d[:, :cfd], Act.Relu,
                    bias=bias127[:], scale=BIGSCALE,
                )
                out_w += cfd
                if h % 2 == 1 or h == len(hchunks) - 1:
                    nc.sync.dma_start(
                        y[:, out_f0 * fd : out_f0 * fd + out_w], out[:, :out_w]
                    )
                prev_last = cur[:, cfd - fd : cfd]
    nc.finalize()
    return nc


_NC_CACHE: dict[str, object] = {}


def _get_nc():
    if "nc" not in _NC_CACHE:
        _NC_CACHE["nc"] = build_nc()
    return _NC_CACHE["nc"]


_NOISE_ORDER = [(i + 1) % N_NOISE for i in range(N_NOISE)]  # storage pos -> plane


def make_nb(on_noise, off_noise):
    """Host-side noise-bias planes: [10, H, W] bf16 in storage order."""
    import ml_dtypes

    on_f = np.asarray(on_noise)[_NOISE_ORDER].astype(np.float32)
    off_f = np.asarray(off_noise)[_NOISE_ORDER].astype(np.float32)
    return (on_f * NB_ON - off_f * NB_OFF).astype(ml_dtypes.bfloat16)


def make_in_maps(tensor, state, on_noise, off_noise):
    tensor = np.asarray(tensor, dtype=np.float32)
    state = np.asarray(state, dtype=np.float32)
    nb = make_nb(on_noise, off_noise)  # [10, H, W] bf16

    in_maps = []
    for c in range(N_CORES):
        r0, r1 = c * ROWS, (c + 1) * ROWS
        xs = (
            tensor[:, r0:r1, :]
            .reshape(T, P, FD)
            .transpose(1, 0, 2)
            .reshape(P, T * FD)
        )
        nbs = (
            nb[:, r0:r1, :]
            .reshape(N_NOISE, P, FD)
            .transpose(1, 0, 2)
            .reshape(P, N_NOISE * FD)
        )
        in_maps.append(
            {
                "x": np.ascontiguousarray(xs),
                "state": np.ascontiguousarray(state[r0:r1]).reshape(P, FD),
                "nb_m": np.ascontiguousarray(nbs),
            }
        )
    return in_maps


def gather_output(results):
    shards = []
    for c in range(N_CORES):
        yc = np.asarray(results[c]["y"])  # [P, T*FD] u8
        yc = yc.reshape(P, T, FD).transpose(1, 0, 2).reshape(T, ROWS, W)
        shards.append(yc)
    return np.concatenate(shards, axis=1).astype(np.float32)


def kernel(tensor, state, timesurface=None, on_noise=None, off_noise=None, **_kw):
    in_maps = make_in_maps(tensor, state, on_noise, off_noise)
    nc = _get_nc()
    res = run_bass_kernel_spmd(nc, in_maps, core_ids=list(range(N_CORES)))
    return gather_output(res.results)
